# revision 1
# baseline (speedup 1.0000x reference)
"""Fused multi-head tanh-attention kernel for Trainium2 (8 NeuronCores).

Problem: y[s,b,:] = concat_h( softmax_t(tanh(q_h k_h^T / 8) - 10000*(1-mask)) @ v_h )
with q/k/v = per-head projections of x.  Shapes: x [1024,16,512], mask [16,1024],
w* [8,64,512] -> y [1024,16,512].

Strategy: batch-parallel over 8 cores (2 batches per core).  Per core, a fully
fused flash-style pipeline keeps the [S,S] score matrices in PSUM/SBUF only:
  - masked keys are COMPACTED away: the host computes, per batch, the index
    list of valid keys (mask=1) and the kernel gathers those x rows via
    indirect DMA, so the key dimension shrinks from 8 to ceil(nb/128) (=7
    here) 128-chunks.  Padding slots carry a compacted mask of 0 and are
    zeroed exactly like reference's exp(-10000) == 0 underflow,
  - x and the gathered x are transposed on-chip via PE-transpose (contraction
    dim on partitions); all matmuls run in float32r (full PE rate, ~1e-4 rel
    err),
  - scores are built in scoresT [t,s] layout; the compacted mask is folded
    into the v tiles (v rows and the appended ones-columns are scaled by it),
  - tanh+exp run on the scalar engine (one table set holds both; exp covers
    two t-chunks per instruction), PV accumulates unnormalized out^T plus the
    softmax denominator (ones-column trick), which is PE-transposed back and
    divided on the vector engine,
  - the scalar engine is the bottleneck, so all other work (projections, v
    construction, gathers, the previous head's epilogue, weight transposes)
    is emitted interleaved into the attention t-loop as background tasks so
    the in-order engine queues never starve the activation engine.
"""

import sys

sys.path.insert(0, "/opt/trn_rl_repo")

from contextlib import ExitStack

import numpy as np

S, B, D, H, DH = 1024, 16, 512, 8, 64
NCORES = 8
BPC = B // NCORES  # batches per core
SC = S // 128  # 8 query chunks
DC = D // 128  # 4 d-chunks
DEFAULT_NKC = 7  # key chunks after mask compaction (ceil(max_nb/128))

_compiled = {}


def _nsplits(total):
    out, p = [], 0
    while p < total:
        sz = min(512, total - p)
        out.append((p, sz))
        p += sz
    return out


def _groups(nkc):
    gs, i = [], 0
    while i < nkc:
        gs.append(tuple(range(i, min(i + 2, nkc))))
        i += 2
    return gs


def _make_pools(tc, ctx):
    pools = {}
    pools["singles"] = ctx.enter_context(tc.tile_pool(name="singles", bufs=1))
    pools["nat"] = ctx.enter_context(tc.tile_pool(name="nat", bufs=3))
    pools["qk"] = ctx.enter_context(tc.tile_pool(name="qk", bufs=4))
    pools["vh"] = ctx.enter_context(tc.tile_pool(name="vh", bufs=8))
    pools["tanh"] = ctx.enter_context(tc.tile_pool(name="tanh", bufs=3))
    pools["exp"] = ctx.enter_context(tc.tile_pool(name="exp", bufs=3))
    pools["outT"] = ctx.enter_context(tc.tile_pool(name="outT", bufs=2))
    pools["outsb"] = ctx.enter_context(tc.tile_pool(name="outsb", bufs=2))
    pools["small"] = ctx.enter_context(tc.tile_pool(name="small", bufs=4))
    # PSUM: 8 banks.  ps_big ([128,1024]f32 slots = 2 banks, bufs=3 = 6
    # banks) rotates scores / projections / v chunks / all transposes.
    # ps_o (2 banks, bufs=1) holds the per-head PV accumulator.
    pools["ps_big"] = ctx.enter_context(
        tc.tile_pool(name="ps_big", bufs=3, space="PSUM")
    )
    pools["ps_o"] = ctx.enter_context(tc.tile_pool(name="ps_o", bufs=1, space="PSUM"))
    return pools


def _emit(nc, tc, pools, tile, mybir, bass, aps, nkc, u=0):
    f32 = mybir.dt.float32
    f32r = mybir.dt.float32r
    i32 = mybir.dt.int32
    AF = mybir.ActivationFunctionType
    Alu = mybir.AluOpType
    x_d, kidx_d, kmsk_d, wq_d, wk_d, wv_d, id_d, y_d = aps
    NK = nkc * 128

    singles = pools["singles"]
    nat = pools["nat"]
    qk_pool = pools["qk"]
    vh_pool = pools["vh"]
    tanh_pool = pools["tanh"]
    exp_pool = pools["exp"]
    outT_pool = pools["outT"]
    outsb_pool = pools["outsb"]
    small = pools["small"]
    ps_big = pools["ps_big"]
    ps_o = pools["ps_o"]

    yr = y_d.rearrange("(c p) b e -> p c b e", p=128)
    xflat = x_d.rearrange("s b d -> (s b) d")

    # ---------------- prologue ------------------------------------------
    ident = singles.tile([128, 128], f32r, tag="ident", name=f"ident_u{u}")
    nc.sync.dma_start(ident, id_d)
    fill64 = singles.tile([128, nkc, 64], f32, tag="fill64", name=f"fill64_u{u}")
    nc.vector.memset(fill64, 1.0)

    # compacted key indices / mask columns, per batch
    kidx = {}
    kmsk = {}
    for b in range(BPC):
        ki = singles.tile([128, nkc], i32, tag=f"kidx{b}", name=f"kidx{b}_u{u}")
        nc.sync.dma_start(ki, kidx_d[b].rearrange("(c p) -> p c", p=128))
        kidx[b] = ki
        km = small.tile([128, nkc], f32, tag="msk", name=f"kmsk{b}_u{u}")
        nc.sync.dma_start(km, kmsk_d[b].rearrange("(c p) -> p c", p=128))
        kmsk[b] = km
        mf = singles.tile([128, nkc, 64], f32, tag=f"mfill{b}", name=f"mfill{b}_u{u}")
        for tck in range(nkc):
            nc.vector.tensor_scalar(
                mf[:, tck, :], fill64[:, tck, :], km[:, tck : tck + 1], None, Alu.mult
            )
        kmsk[b, "fill"] = mf

    # ---- emitters ------------------------------------------------------
    wTq = {}
    wTk = {}
    wTv = {}

    def emit_wqk_tr(nm, w_d, wT, hp):
        w_nat = nat.tile([128, D], f32r, tag="nat", name=f"w_nat_u{u}")
        nc.sync.dma_start(w_nat, w_d[2 * hp : 2 * hp + 2].rearrange("h e d -> (h e) d"))
        wt = singles.tile(
            [128, DC, 128], f32r, tag=f"wT{nm}{hp}", name=f"wT{nm}{hp}_u{u}"
        )
        wT[hp] = wt
        for dc in range(DC):
            pst = ps_big.tile([128, 128], f32r, tag="ps_big", name=f"pstr_u{u}")
            nc.tensor.transpose(pst, w_nat[:, dc * 128 : dc * 128 + 128], ident)
            nc.vector.tensor_copy(wt[:, dc, :], pst)

    def emit_wv_tr(q4):
        wt = singles.tile([128, DC, 256], f32r, tag=f"wTv{q4}", name=f"wTv{q4}_u{u}")
        wTv[q4] = wt
        for half in range(2):
            w_nat = nat.tile([128, D], f32r, tag="nat", name=f"w_nat_u{u}")
            h0 = 4 * q4 + 2 * half
            nc.sync.dma_start(w_nat, wv_d[h0 : h0 + 2].rearrange("h e d -> (h e) d"))
            for dc in range(DC):
                pst = ps_big.tile([128, 128], f32r, tag="ps_big", name=f"pstr_u{u}")
                nc.tensor.transpose(pst, w_nat[:, dc * 128 : dc * 128 + 128], ident)
                nc.vector.tensor_copy(wt[:, dc, half * 128 : half * 128 + 128], pst)

    xbT = {}  # full x^T (query side)
    xkT = {}  # gathered x^T (key side)

    def alloc_xbT(b):
        for dc in range(DC):
            xbT[b, dc] = singles.tile(
                [128, S], f32r, tag=f"xbT{b}{dc}", name=f"xbT{b}{dc}_u{u}"
            )

    def alloc_xkT(b):
        for dc in range(DC):
            xkT[b, dc] = singles.tile(
                [128, NK], f32r, tag=f"xkT{b}{dc}", name=f"xkT{b}{dc}_u{u}"
            )

    def emit_x_tr(b, sc):
        x_nat = nat.tile([128, D], f32r, tag="nat", name=f"x_nat_u{u}")
        nc.sync.dma_start(x_nat, x_d[sc * 128 : sc * 128 + 128, b, :])
        for dc in range(DC):
            pst = ps_big.tile([128, 128], f32r, tag="ps_big", name=f"pstr_u{u}")
            nc.tensor.transpose(pst, x_nat[:, dc * 128 : dc * 128 + 128], ident)
            nc.vector.tensor_copy(xbT[b, dc][:, sc * 128 : sc * 128 + 128], pst)

    def emit_xk_dma(b, c):
        xk_nat = nat.tile([128, D], f32r, tag="xknat", name=f"xk_nat_u{u}")
        nc.gpsimd.indirect_dma_start(
            out=xk_nat,
            out_offset=None,
            in_=xflat,
            in_offset=bass.IndirectOffsetOnAxis(ap=kidx[b][:, c : c + 1], axis=0),
        )
        return xk_nat

    def emit_xk_tr(b, c, xk_nat):
        for dc in range(DC):
            pst = ps_big.tile([128, 128], f32r, tag="ps_big", name=f"pstr_u{u}")
            nc.tensor.transpose(pst, xk_nat[:, dc * 128 : dc * 128 + 128], ident)
            nc.vector.tensor_copy(xkT[b, dc][:, c * 128 : c * 128 + 128], pst)

    def emit_xk_gather(b, c):
        emit_xk_tr(b, c, emit_xk_dma(b, c))

    qkT = {}

    def _proj_half(b, hp, nm, off, sz):
        wT, src = (wTq, xbT) if nm == "q" else (wTk, xkT)
        psp = ps_big.tile([128, 512], f32, tag="ps_big", name=f"psp_u{u}")
        for dc in range(DC):
            nc.tensor.matmul(
                psp[:, 0:sz],
                wT[hp][:, dc, :],
                src[b, dc][:, off : off + sz],
                start=(dc == 0),
                stop=(dc == DC - 1),
            )
        nc.vector.tensor_copy(qkT[b, hp, nm][:, off : off + sz], psp[:, 0:sz])

    def _alloc_qkT(b, hp, nm):
        t = qk_pool.tile([128, S], f32r, tag="qkT", name=f"qkT{nm}_u{u}")
        qkT[b, hp, nm] = t

    def emit_proj_q(b, hp, half=None):
        if (b, hp, "q") not in qkT:
            _alloc_qkT(b, hp, "q")
        splits = _nsplits(S)
        for i, (off, sz) in enumerate(splits):
            if half is None or half == i:
                _proj_half(b, hp, "q", off, sz)

    def emit_proj_k(b, hp, half=None):
        if (b, hp, "k") not in qkT:
            _alloc_qkT(b, hp, "k")
        splits = _nsplits(NK)
        for i, (off, sz) in enumerate(splits):
            if half is None or half == i:
                _proj_half(b, hp, "k", off, sz)

    vh = {}

    def alloc_vh(b, q4):
        for h in range(4 * q4, 4 * q4 + 4):
            vh[b, h] = vh_pool.tile(
                [128, nkc, 128], f32r, tag="vh", name=f"vh{b}_{h}_u{u}"
            )
            nc.vector.tensor_copy(vh[b, h][:, :, 64:128], kmsk[b, "fill"])

    def emit_v_chunk(b, q4, tck):
        psv = ps_big.tile([128, 256], f32, tag="ps_big", name=f"psv_u{u}")
        for dc in range(DC):
            nc.tensor.matmul(
                psv,
                xkT[b, dc][:, tck * 128 : tck * 128 + 128],
                wTv[q4][:, dc, :],
                start=(dc == 0),
                stop=(dc == DC - 1),
            )
        for h_in, h in enumerate(range(4 * q4, 4 * q4 + 4)):
            nc.vector.tensor_scalar(
                vh[b, h][:, tck, 0:64],
                psv[:, h_in * 64 : h_in * 64 + 64],
                kmsk[b][:, tck : tck + 1],
                None,
                Alu.mult,
            )

    def out_stage_parts(b, h, pso):
        state = {}

        def p1():
            outT = outT_pool.tile([128, S], f32r, tag="outT", name=f"outT_u{u}")
            nc.vector.tensor_copy(outT, pso)
            state["outT"] = outT

        def p2():
            pst = ps_big.tile([128, SC, 128], f32r, tag="ps_big", name=f"psto_u{u}")
            for sc in range(SC):
                nc.tensor.transpose(
                    pst[:, sc, 0:128],
                    state["outT"][:, sc * 128 : sc * 128 + 128],
                    ident,
                )
            state["pst"] = pst

        def p3():
            pst = state["pst"]
            rec = small.tile([128, SC], f32, tag="rec", name=f"rec_u{u}")
            nc.vector.reciprocal(rec, pst[:, :, 64])
            osb = outsb_pool.tile([128, SC, 64], f32, tag="osb", name=f"osb_u{u}")
            for sc in range(SC):
                nc.vector.tensor_scalar(
                    osb[:, sc, :], pst[:, sc, 0:64], rec[:, sc : sc + 1], None, Alu.mult
                )
            nc.sync.dma_start(yr[:, :, b, h * 64 : h * 64 + 64], osb)

        return [p1, p2, p3]

    # ---------------- bootstrap -----------------------------------------
    alloc_xbT(0)
    alloc_xkT(0)
    # q-projection chain first: it gates the first QK (needs ALL of x^T),
    # so nothing may sit ahead of it in the PE stream or the DMA queues.
    emit_wqk_tr("q", wq_d, wTq, 0)
    for sc in range(SC):
        emit_x_tr(0, sc)
    emit_proj_q(0, 0)
    xk_nats = [emit_xk_dma(0, c) for c in range(nkc)]
    emit_wqk_tr("k", wk_d, wTk, 0)
    for c in range(nkc):
        emit_xk_tr(0, c, xk_nats[c])
    emit_proj_k(0, 0)
    emit_wv_tr(0)
    alloc_vh(0, 0)
    for tck in range(4):
        emit_v_chunk(0, 0, tck)

    # background task lists per head index
    NH = BPC * H
    bg = {i: [] for i in range(NH + 1)}
    bg[0] += [(lambda tck=tck: emit_v_chunk(0, 0, tck)) for tck in range(4, nkc)]
    bg[0] += [
        (lambda hp=hp: emit_wqk_tr("q", wq_d, wTq, hp)) for hp in range(1, H // 2)
    ]
    bg[1] += [
        (lambda hp=hp: emit_wqk_tr("k", wk_d, wTk, hp)) for hp in range(1, H // 2)
    ]
    bg[1] += [lambda: emit_wv_tr(1)]
    for b in range(BPC):
        base = b * H
        for h in range(1, H, 2):
            if h < H - 1:
                hp = (h + 1) // 2
                bg[base + h] += [
                    lambda b=b, hp=hp: emit_proj_q(b, hp, 0),
                    lambda b=b, hp=hp: emit_proj_q(b, hp, 1),
                    lambda b=b, hp=hp: emit_proj_k(b, hp, 0),
                    lambda b=b, hp=hp: emit_proj_k(b, hp, 1),
                ]
        bg[base + 2] += [lambda b=b: alloc_vh(b, 1)]
        bg[base + 2] += [
            (lambda b=b, tck=tck: emit_v_chunk(b, 1, tck)) for tck in range(0, 4)
        ]
        bg[base + 3] += [
            (lambda b=b, tck=tck: emit_v_chunk(b, 1, tck)) for tck in range(4, nkc)
        ]
    if BPC > 1:
        bg[4] += [lambda: alloc_xbT(1), lambda: alloc_xkT(1)]
        bg[4] += [(lambda sc=sc: emit_x_tr(1, sc)) for sc in range(0, 3)]
        bg[5] += [(lambda sc=sc: emit_x_tr(1, sc)) for sc in range(3, 6)]
        bg[5] += [(lambda c=c: emit_xk_gather(1, c)) for c in range(0, 3)]
        bg[6] += [(lambda sc=sc: emit_x_tr(1, sc)) for sc in range(6, SC)]
        bg[6] += [(lambda c=c: emit_xk_gather(1, c)) for c in range(3, nkc)]
        bg[7] += [
            lambda: emit_proj_q(1, 0),
            lambda: emit_proj_k(1, 0),
            lambda: alloc_vh(1, 0),
        ]
        bg[7] += [(lambda tck=tck: emit_v_chunk(1, 0, tck)) for tck in range(nkc)]

    # ---------------- main attention loop --------------------------------
    heads = [(b, h) for b in range(BPC) for h in range(H)]
    groups = _groups(nkc)

    def emit_qk_chunk(b, hp, h2, tck):
        r0 = h2 * 64
        kT = qkT[b, hp, "k"]
        qT = qkT[b, hp, "q"]
        pss = ps_big.tile([128, S], f32, tag="ps_big", name=f"pss_u{u}")
        for sh in range(2):
            nc.tensor.matmul(
                pss[:, sh * 512 : sh * 512 + 512],
                kT[r0 : r0 + 64, tck * 128 : tck * 128 + 128],
                qT[r0 : r0 + 64, sh * 512 : sh * 512 + 512],
                start=True,
                stop=True,
            )
        return pss

    # flat schedule of (head-index, group) so each group's first QK can be
    # emitted one group early (before the previous group's PVs), keeping the
    # activation engine from waiting on the in-order PE queue.
    sched = []
    for hi in range(len(heads)):
        for gi, grp in enumerate(groups):
            sched.append((hi, gi, grp))
    pending_qk = None
    pso = None
    for si, (hi, gi, grp) in enumerate(sched):
        b, h = heads[hi]
        hp, h2 = h // 2, h % 2
        if gi == 0:
            pso = ps_o.tile([128, S], f32, tag="ps_o", name=f"pso_u{u}")
            pso_by_head = getattr(emit_qk_chunk, "_pso", {})
            pso_by_head[hi] = pso
            emit_qk_chunk._pso = pso_by_head
            tasks = list(bg[hi])
            done = 0
        tnh = tanh_pool.tile([128, len(grp), S], f32, tag="tanh", name=f"tnh_u{u}")
        for j, tck in enumerate(grp):
            if j == 0 and pending_qk is not None:
                pss = pending_qk
                pending_qk = None
            else:
                pss = emit_qk_chunk(b, hp, h2, tck)
            nc.scalar.activation(tnh[:, j, :], pss, AF.Tanh, scale=0.125)
        ex = exp_pool.tile([128, len(grp), S], f32r, tag="exp", name=f"ex_u{u}")
        nc.scalar.activation(ex, tnh.rearrange("p a s -> p (a s)"), AF.Exp)
        # drain background work (keeps PE/DVE busy while ACT runs)
        target = (len(tasks) * (gi + 1) + len(groups) - 1) // len(groups)
        while done < target:
            tasks[done]()
            done += 1
        # emit the NEXT group's first QK before this group's PVs
        if si + 1 < len(sched):
            nhi, ngi, ngrp = sched[si + 1]
            nb_, nh_ = heads[nhi]
            pending_qk = emit_qk_chunk(nb_, nh_ // 2, nh_ % 2, ngrp[0])
        for j, tck in enumerate(grp):
            for sh in range(2):
                nc.tensor.matmul(
                    pso[:, sh * 512 : sh * 512 + 512],
                    vh[b, h][:, tck, :],
                    ex[:, j, sh * 512 : sh * 512 + 512],
                    start=(tck == 0),
                    stop=(tck == nkc - 1),
                )
        if gi == len(groups) - 1:
            bg[hi + 1] = out_stage_parts(b, h, pso) + bg[hi + 1]
    for t in bg[NH]:
        t()


def _build(unroll=1, nkc=DEFAULT_NKC):
    import concourse.bass as bass
    import concourse.tile as tile
    from concourse import bacc, mybir

    f32 = mybir.dt.float32
    f32r = mybir.dt.float32r
    i32 = mybir.dt.int32
    NK = nkc * 128
    nc = bacc.Bacc("TRN2", target_bir_lowering=False, debug=False)
    x_d = nc.dram_tensor("x", [S, BPC, D], f32r, kind="ExternalInput").ap()
    kidx_d = nc.dram_tensor("kidx", [BPC, NK], i32, kind="ExternalInput").ap()
    kmsk_d = nc.dram_tensor("kmsk", [BPC, NK], f32, kind="ExternalInput").ap()
    wq_d = nc.dram_tensor("wq", [H, DH, D], f32r, kind="ExternalInput").ap()
    wk_d = nc.dram_tensor("wk", [H, DH, D], f32r, kind="ExternalInput").ap()
    wv_d = nc.dram_tensor("wv", [H, DH, D], f32r, kind="ExternalInput").ap()
    id_d = nc.dram_tensor("ident", [128, 128], f32r, kind="ExternalInput").ap()
    y_d = nc.dram_tensor("y", [S, BPC, D], f32, kind="ExternalOutput").ap()
    with tile.TileContext(nc) as tc, ExitStack() as ctx:
        pools = _make_pools(tc, ctx)
        aps = (x_d, kidx_d, kmsk_d, wq_d, wk_d, wv_d, id_d, y_d)
        for u in range(unroll):
            _emit(nc, tc, pools, tile, mybir, bass, aps, nkc, u)
    nc.compile()
    return nc


def get_compiled(nkc=DEFAULT_NKC):
    if nkc not in _compiled:
        _compiled[nkc] = _build(nkc=nkc)
    return _compiled[nkc]


def _compute_nkc(mask):
    nb_max = int((np.asarray(mask) != 0).sum(axis=1).max())
    return max(1, -(-nb_max // 128))


def make_in_maps(x, mask, wq, wk, wv, nkc=DEFAULT_NKC):
    x = np.asarray(x, np.float32)
    mask = np.asarray(mask, np.float32)
    wq = np.ascontiguousarray(np.asarray(wq, np.float32))
    wk = np.ascontiguousarray(np.asarray(wk, np.float32))
    wv = np.ascontiguousarray(np.asarray(wv, np.float32))
    ident = np.eye(128, dtype=np.float32)
    NK = nkc * 128
    maps = []
    for c in range(NCORES):
        mb = mask[c * BPC : (c + 1) * BPC, :]
        kidx = np.zeros((BPC, NK), np.int32)
        kmsk = np.zeros((BPC, NK), np.float32)
        for b in range(BPC):
            valid = np.nonzero(mb[b] != 0)[0][:NK]
            # row index into the per-core x flattened as [(s b), d]
            kidx[b, : len(valid)] = valid.astype(np.int32) * BPC + b
            kmsk[b, : len(valid)] = 1.0
        maps.append(
            {
                "x": np.ascontiguousarray(x[:, c * BPC : (c + 1) * BPC, :]),
                "kidx": kidx,
                "kmsk": kmsk,
                "wq": wq,
                "wk": wk,
                "wv": wv,
                "ident": ident,
            }
        )
    return maps


def kernel(x, mask, wq, wk, wv):
    from concourse.bass_utils import run_bass_kernel_spmd

    nkc = _compute_nkc(mask)
    nc = get_compiled(nkc)
    in_maps = make_in_maps(x, mask, wq, wk, wv, nkc=nkc)
    res = run_bass_kernel_spmd(nc, in_maps, list(range(NCORES))).results
    y = np.concatenate([r["y"] for r in res], axis=1)
    return np.ascontiguousarray(y.astype(np.float32, copy=False))



# revision 7
# speedup vs baseline: 1.3228x; 1.3228x over previous
"""Fused multi-head tanh-attention kernel for Trainium2 (8 NeuronCores).

Problem: y[s,b,:] = concat_h( softmax_t(tanh(q_h k_h^T / 8) - 10000*(1-mask)) @ v_h )
with q/k/v = per-head projections of x.  Shapes: x [1024,16,512], mask [16,1024],
w* [8,64,512] -> y [1024,16,512].

Strategy: batch-parallel over 8 cores (2 batches per core).  Per core, a fully
fused flash-style pipeline keeps the [S,S] score matrices in PSUM/SBUF only.

Key optimizations over the tanh+exp two-pass baseline:
  - softmax numerator exp(tanh(z)) is replaced by the fitted d + a*sigmoid(b*z+c)
    (max rel err 0.75%): ONE activation pass instead of two halves the ACT
    engine load, which was the bottleneck.  The softmax ratio cancels `a`; the
    offset `d` is restored exactly via a per-head rank-1 PSUM accumulation
    (d/a * [sum_t v_t | nb]) computed from tiny column-sum matmuls.
  - masked keys are COMPACTED away host-side (indirect DMA gather), key dim
    shrinks to nkc=ceil(nb_max/128) 128-chunks; padding rows carry zeroed v.
  - x and w arrive in bf16; ALL transposes (x, gathered x, weights, out^T) run
    on the DMA XBAR (dma_start_transpose), none on the PE.
  - probabilities and v are fp8e4: PV runs DoubleRow matmuls (2 key chunks per
    instruction at 0.5 cyc/row).  QK runs bf16.
  - epilogue: Pool engine copies PSUM out^T to SBUF bf16, DMA transposes it,
    DVE normalizes by the denominator column and streams y out.
"""

import sys

sys.path.insert(0, "/opt/trn_rl_repo")

from contextlib import ExitStack

import numpy as np

S, B, D, H, DH = 1024, 16, 512, 8, 64
NCORES = 8
BPC = B // NCORES  # batches per core
SC = S // 128  # 8 query chunks
DC = D // 128  # 4 d-chunks
DEFAULT_NKC = 7  # key chunks after mask compaction (ceil(max_nb/128))

# exp(tanh(z)) ~= FD + FA * sigmoid(FB*z + FC), z = scores/8
FA = 2.32729628
FB = 2.15960625
FC = -0.99248019
FD = 0.3706459
SCALE = FB / 8.0
DA = FD / FA

NF = 80  # vh columns: 64 v dims + 16 mask-fill (denominator) columns

_compiled = {}


def _ksplits(total):
    out, p = [], 0
    while p < total:
        sz = min(512, total - p)
        out.append((p, sz))
        p += sz
    return out


def _make_pools(tc, ctx):
    pools = {}
    pools["singles"] = ctx.enter_context(tc.tile_pool(name="singles", bufs=1))
    pools["xknat"] = ctx.enter_context(tc.tile_pool(name="xknat", bufs=3))
    pools["ex"] = ctx.enter_context(tc.tile_pool(name="ex", bufs=3))
    pools["vrow"] = ctx.enter_context(tc.tile_pool(name="vrow", bufs=3))
    pools["outsb"] = ctx.enter_context(tc.tile_pool(name="outsb", bufs=2))
    pools["pst"] = ctx.enter_context(tc.tile_pool(name="pst", bufs=2))
    pools["osb"] = ctx.enter_context(tc.tile_pool(name="osb", bufs=2))
    pools["small"] = ctx.enter_context(tc.tile_pool(name="small", bufs=4))
    # PSUM: 8 banks.  ps_big ([128,1024]f32 slots = 2 banks, bufs=3 = 6 banks)
    # rotates QK scores / projections / v chunks / vsum.  ps_o (2 banks,
    # bufs=1) holds the per-head PV accumulator [NF, S].
    pools["ps_big"] = ctx.enter_context(
        tc.tile_pool(name="ps_big", bufs=3, space="PSUM")
    )
    pools["ps_o"] = ctx.enter_context(tc.tile_pool(name="ps_o", bufs=1, space="PSUM"))
    return pools


def _emit(nc, tc, pools, tile, mybir, bass, aps, nkc, u=0):
    f32 = mybir.dt.float32
    bf16 = mybir.dt.bfloat16
    fp8 = mybir.dt.float8e4
    i32 = mybir.dt.int32
    AF = mybir.ActivationFunctionType
    Alu = mybir.AluOpType
    DR = mybir.MatmulPerfMode.DoubleRow
    x_d, kidx_d, kmsk_d, wq_d, wk_d, wv_d, y_d = aps
    NK = nkc * 128
    npair = nkc // 2  # DoubleRow chunk pairs; one single chunk if nkc odd

    singles = pools["singles"]
    xknat_pool = pools["xknat"]
    ex_pool = pools["ex"]
    vrow_pool = pools["vrow"]
    outsb_pool = pools["outsb"]
    pst_pool = pools["pst"]
    osb_pool = pools["osb"]
    small = pools["small"]
    ps_big = pools["ps_big"]
    ps_o = pools["ps_o"]

    yr = y_d.rearrange("(c p) b e -> p c b e", p=128)
    xflat = x_d.rearrange("s b d -> (s b) d")

    # ---------------- shared constants ----------------------------------
    ones_col8 = singles.tile([128, 1], fp8, tag="ones8", name=f"ones8_u{u}")
    nc.vector.memset(ones_col8, 1.0)
    da_row = singles.tile([1, 512], bf16, tag="da_row", name=f"da_row_u{u}")
    nc.vector.memset(da_row, DA)
    bias_c = singles.tile([128, 1], f32, tag="bias_c", name=f"bias_c_u{u}")
    nc.vector.memset(bias_c, FC)
    fill16 = singles.tile([128, nkc, 16], f32, tag="fill16", name=f"fill16_u{u}")
    nc.vector.memset(fill16, 1.0)

    # compacted key indices / mask columns, per batch
    kidx = {}
    kmsk = {}
    mfill = {}
    for b in range(BPC):
        ki = singles.tile([128, nkc], i32, tag=f"kidx{b}", name=f"kidx{b}_u{u}")
        nc.sync.dma_start(ki, kidx_d[b].rearrange("(c p) -> p c", p=128))
        kidx[b] = ki
        km = singles.tile([128, nkc], f32, tag=f"kmsk{b}", name=f"kmsk{b}_u{u}")
        nc.sync.dma_start(km, kmsk_d[b].rearrange("(c p) -> p c", p=128))
        kmsk[b] = km

    def emit_mfill(b):
        mf = singles.tile([128, nkc, 16], fp8, tag=f"mfill{b}", name=f"mfill{b}_u{u}")
        for tck in range(nkc):
            nc.vector.tensor_scalar(
                mf[:, tck, :], fill16[:, tck, :], kmsk[b][:, tck : tck + 1], None,
                Alu.mult,
            )
        mfill[b] = mf

    # ---- DMA-transposed inputs -----------------------------------------
    wTq = {}
    wTk = {}
    wTv = {}

    def emit_wqk_tr(nm, w_d, wT, hp):
        wt = singles.tile([128, DC, 128], bf16, tag=f"wT{nm}{hp}", name=f"wT{nm}{hp}_u{u}")
        wT[hp] = wt
        nc.sync.dma_start_transpose(
            wt, w_d[2 * hp : 2 * hp + 2].rearrange("h e d -> (h e) d")
        )

    def emit_wv_tr(q4):
        wt = singles.tile([128, DC, 256], bf16, tag=f"wTv{q4}", name=f"wTv{q4}_u{u}")
        wTv[q4] = wt
        nc.sync.dma_start_transpose(
            wt, wv_d[4 * q4 : 4 * q4 + 4].rearrange("h e d -> (h e) d")
        )

    xbT = {}  # full x^T (query side)   [128, DC, S] bf16
    xkT = {}  # gathered x^T (key side) [128, DC, NK] bf16

    def emit_xbT(b):
        t = singles.tile([128, DC, S], bf16, tag=f"xbT{b}", name=f"xbT{b}_u{u}")
        xbT[b] = t
        nc.sync.dma_start_transpose(t, x_d[:, b, :])

    def alloc_xkT(b):
        xkT[b] = singles.tile([128, DC, NK], bf16, tag=f"xkT{b}", name=f"xkT{b}_u{u}")

    def emit_xk_dma(b, c):
        xk_nat = xknat_pool.tile([128, 512], bf16, tag="xknat", name=f"xk_nat_u{u}")
        nc.gpsimd.indirect_dma_start(
            out=xk_nat,
            out_offset=None,
            in_=xflat,
            in_offset=bass.IndirectOffsetOnAxis(ap=kidx[b][:, c : c + 1], axis=0),
        )
        return xk_nat

    def emit_xk_tr(b, c, xk_nat):
        nc.sync.dma_start_transpose(
            xkT[b][:, :, c * 128 : c * 128 + 128], xk_nat
        )

    def emit_xk_gather(b, c):
        emit_xk_tr(b, c, emit_xk_dma(b, c))

    # ---- projections ----------------------------------------------------
    qkT = {}

    def _proj_half(b, hp, nm, off, sz):
        wT, src = (wTq, xbT) if nm == "q" else (wTk, xkT)
        psp = ps_big.tile([128, 512], f32, tag="ps_big", name=f"psp_u{u}")
        for dc in range(DC):
            nc.tensor.matmul(
                psp[:, 0:sz],
                wT[hp][:, dc, :],
                src[b][:, dc, off : off + sz],
                start=(dc == 0),
                stop=(dc == DC - 1),
            )
        nc.vector.tensor_copy(qkT[b, hp, nm][:, off : off + sz], psp[:, 0:sz])

    def _alloc_qkT(b, hp, nm):
        n = S if nm == "q" else NK
        qkT[b, hp, nm] = singles.tile(
            [128, n], bf16, tag=f"qkT{nm}{b}{hp}", name=f"qkT{nm}{b}{hp}_u{u}"
        )

    def emit_proj(b, hp, nm, half=None):
        if (b, hp, nm) not in qkT:
            _alloc_qkT(b, hp, nm)
        splits = _ksplits(S if nm == "q" else NK)
        for i, (off, sz) in enumerate(splits):
            if half is None or half == i:
                _proj_half(b, hp, nm, off, sz)

    # ---- v construction --------------------------------------------------
    # vh8[b]: [128, nkc, H, NF] fp8; cols 0:64 = masked v, 64:NF = kmsk fill
    vh8 = {}

    def alloc_vh(b):
        vh8[b] = singles.tile(
            [128, nkc, H, NF], fp8, tag=f"vh{b}", name=f"vh{b}_u{u}"
        )

    def emit_vh_fill(b, h):
        nc.vector.tensor_copy(vh8[b][:, :, h, 64:NF], mfill[b])

    def emit_v_chunk(b, q4, tck):
        psv = ps_big.tile([128, 256], f32, tag="ps_big", name=f"psv_u{u}")
        for dc in range(DC):
            nc.tensor.matmul(
                psv,
                xkT[b][:, dc, tck * 128 : tck * 128 + 128],
                wTv[q4][:, dc, :],
                start=(dc == 0),
                stop=(dc == DC - 1),
            )
        for h_in, h in enumerate(range(4 * q4, 4 * q4 + 4)):
            nc.vector.tensor_scalar(
                vh8[b][:, tck, h, 0:64],
                psv[:, h_in * 64 : h_in * 64 + 64],
                kmsk[b][:, tck : tck + 1],
                None,
                Alu.mult,
            )

    # ---- per-head offset correction (d/a * [sum_t v | nb]) ---------------
    vrow = {}

    def emit_vsum(b, h):
        vsr = ps_big.tile([1, NF], f32, tag="ps_big", name=f"vsr_u{u}")
        for tck in range(nkc):
            nc.tensor.matmul(
                vsr,
                ones_col8,
                vh8[b][:, tck, h, :],
                start=(tck == 0),
                stop=(tck == nkc - 1),
            )
        vr = vrow_pool.tile([1, NF], bf16, tag="vrow", name=f"vrow_u{u}")
        nc.vector.tensor_copy(vr, vsr)
        vrow[b, h] = vr

    # ---- epilogue --------------------------------------------------------
    def out_stage_parts(b, h, pso):
        state = {}

        def p1():
            outsb = outsb_pool.tile([NF, S], bf16, tag="outsb", name=f"outsb_u{u}")
            nc.vector.tensor_copy(outsb, pso)
            state["outsb"] = outsb

        def p2():
            pst = pst_pool.tile([128, SC, NF], bf16, tag="pst", name=f"pst_u{u}")
            nc.sync.dma_start_transpose(pst, state["outsb"])
            state["pst"] = pst

        def p3():
            pst = state["pst"]
            rec = small.tile([128, SC], f32, tag="rec", name=f"rec_u{u}")
            nc.vector.reciprocal(rec, pst[:, :, 64])
            osb = osb_pool.tile([128, SC, 64], f32, tag="osb", name=f"osb_u{u}")
            for sc in range(SC):
                nc.gpsimd.tensor_scalar(
                    osb[:, sc, :], pst[:, sc, 0:64], rec[:, sc : sc + 1], None,
                    Alu.mult,
                )
            nc.sync.dma_start(yr[:, :, b, h * 64 : h * 64 + 64], osb)

        return [p1, p2, p3]

    # ---------------- bootstrap -----------------------------------------
    emit_wqk_tr("q", wq_d, wTq, 0)
    emit_xbT(0)
    emit_proj(0, 0, "q")
    alloc_xkT(0)
    xk_nats = [emit_xk_dma(0, c) for c in range(nkc)]
    emit_wqk_tr("k", wk_d, wTk, 0)
    emit_wv_tr(0)
    emit_mfill(0)
    for c in range(nkc):
        emit_xk_tr(0, c, xk_nats[c])
    emit_proj(0, 0, "k")
    alloc_vh(0)
    for h in range(4):
        emit_vh_fill(0, h)
    for tck in range(nkc):
        emit_v_chunk(0, 0, tck)
    emit_vsum(0, 0)

    # background task lists per head index
    NH = BPC * H
    bg = {i: [] for i in range(NH + 1)}
    bg[0] += [
        (lambda hp=hp: emit_wqk_tr("q", wq_d, wTq, hp)) for hp in range(1, H // 2)
    ]
    bg[0] += [
        (lambda hp=hp: emit_wqk_tr("k", wk_d, wTk, hp)) for hp in range(1, H // 2)
    ]
    bg[0] += [lambda: emit_wv_tr(1)]
    for b in range(BPC):
        base = b * H
        for h in range(1, H, 2):
            if h < H - 1:
                hp = (h + 1) // 2
                bg[base + h] += [
                    lambda b=b, hp=hp: emit_proj(b, hp, "q", 0),
                    lambda b=b, hp=hp: emit_proj(b, hp, "q", 1),
                    lambda b=b, hp=hp: emit_proj(b, hp, "k", 0),
                    lambda b=b, hp=hp: emit_proj(b, hp, "k", 1),
                ]
        bg[base + 2] += [(lambda b=b, h=h: emit_vh_fill(b, h)) for h in range(4, 8)]
        bg[base + 2] += [
            (lambda b=b, tck=tck: emit_v_chunk(b, 1, tck)) for tck in range(0, 4)
        ]
        bg[base + 3] += [
            (lambda b=b, tck=tck: emit_v_chunk(b, 1, tck)) for tck in range(4, nkc)
        ]
        # per-head vsum one head ahead (vh for its q4 group must be complete;
        # head (b>0, 0)'s vsum is appended after bg[7]'s v chunks below)
        for h in range(H):
            hi = base + h
            if hi + 1 < NH and (hi + 1) % H != 0:
                bn, hn = (hi + 1) // H, (hi + 1) % H
                bg[hi] += [lambda bn=bn, hn=hn: emit_vsum(bn, hn)]
    if BPC > 1:
        bg[4] += [lambda: emit_xbT(1), lambda: alloc_xkT(1)]
        bg[5] += [(lambda c=c: emit_xk_gather(1, c)) for c in range(0, 4)]
        bg[6] += [(lambda c=c: emit_xk_gather(1, c)) for c in range(4, nkc)]
        bg[6] += [lambda: emit_mfill(1)]
        bg[7] += [
            lambda: emit_proj(1, 0, "q"),
            lambda: emit_proj(1, 0, "k"),
            lambda: alloc_vh(1),
        ]
        bg[7] += [(lambda h=h: emit_vh_fill(1, h)) for h in range(4)]
        bg[7] += [(lambda tck=tck: emit_v_chunk(1, 0, tck)) for tck in range(nkc)]
        bg[7] += [lambda: emit_vsum(1, 0)]

    # ---------------- main attention loop --------------------------------
    heads = [(b, h) for b in range(BPC) for h in range(H)]

    def emit_qk_chunk(b, hp, h2, tck):
        r0 = h2 * 64
        kT = qkT[b, hp, "k"]
        qT = qkT[b, hp, "q"]
        pss = ps_big.tile([128, S], f32, tag="ps_big", name=f"pss_u{u}")
        for sh in range(2):
            nc.tensor.matmul(
                pss[:, sh * 512 : sh * 512 + 512],
                kT[r0 : r0 + 64, tck * 128 : tck * 128 + 128],
                qT[r0 : r0 + 64, sh * 512 : sh * 512 + 512],
                start=True,
                stop=True,
            )
        return pss

    sched = []
    for hi in range(len(heads)):
        for tck in range(nkc):
            sched.append((hi, tck))
    pending_qk = None
    pso = None
    ex8 = None
    for si, (hi, tck) in enumerate(sched):
        b, h = heads[hi]
        hp, h2 = h // 2, h % 2
        if tck == 0:
            pso = ps_o.tile([NF, S], f32, tag="ps_o", name=f"pso_u{u}")
            ex8 = ex_pool.tile([128, nkc, S], fp8, tag="ex", name=f"ex_u{u}")
            tasks = list(bg[hi])
            done = 0
        if pending_qk is not None:
            pss = pending_qk
            pending_qk = None
        else:
            pss = emit_qk_chunk(b, hp, h2, tck)
        nc.scalar.activation(ex8[:, tck, :], pss, AF.Sigmoid, bias=bias_c, scale=SCALE)
        # drain background work (keeps PE/DVE busy while ACT runs)
        target = (len(tasks) * (tck + 1) + nkc - 1) // nkc
        while done < target:
            tasks[done]()
            done += 1
        # emit the NEXT chunk's QK before this chunk's PV
        if si + 1 < len(sched):
            nhi, ntck = sched[si + 1]
            nb_, nh_ = heads[nhi]
            pending_qk = emit_qk_chunk(nb_, nh_ // 2, nh_ % 2, ntck)
        # PV: DoubleRow on completed chunk pairs; single fp8 matmul for the
        # odd final chunk; rank-1 offset correction closes the accumulation.
        if tck % 2 == 1:
            p = tck // 2
            for sh in range(2):
                nc.tensor.matmul(
                    pso[:, sh * 512 : sh * 512 + 512],
                    vh8[b][:, 2 * p : 2 * p + 2, h, :],
                    ex8[:, 2 * p : 2 * p + 2, sh * 512 : sh * 512 + 512],
                    start=(p == 0),
                    stop=False,
                    perf_mode=DR,
                )
        if tck == nkc - 1:
            if nkc % 2 == 1:
                for sh in range(2):
                    nc.tensor.matmul(
                        pso[:, sh * 512 : sh * 512 + 512],
                        vh8[b][:, nkc - 1, h, :],
                        ex8[:, nkc - 1, sh * 512 : sh * 512 + 512],
                        start=(nkc == 1),
                        stop=False,
                    )
            for sh in range(2):
                nc.tensor.matmul(
                    pso[:, sh * 512 : sh * 512 + 512],
                    vrow[b, h],
                    da_row,
                    start=False,
                    stop=True,
                )
            bg[hi + 1] = out_stage_parts(b, h, pso) + bg[hi + 1]
    for t in bg[NH]:
        t()


def _build(unroll=1, nkc=DEFAULT_NKC):
    import concourse.bass as bass
    import concourse.tile as tile
    from concourse import bacc, mybir

    f32 = mybir.dt.float32
    bf16 = mybir.dt.bfloat16
    i32 = mybir.dt.int32
    NK = nkc * 128
    nc = bacc.Bacc("TRN2", target_bir_lowering=False, debug=False)
    x_d = nc.dram_tensor("x", [S, BPC, D], bf16, kind="ExternalInput").ap()
    kidx_d = nc.dram_tensor("kidx", [BPC, NK], i32, kind="ExternalInput").ap()
    kmsk_d = nc.dram_tensor("kmsk", [BPC, NK], f32, kind="ExternalInput").ap()
    wq_d = nc.dram_tensor("wq", [H, DH, D], bf16, kind="ExternalInput").ap()
    wk_d = nc.dram_tensor("wk", [H, DH, D], bf16, kind="ExternalInput").ap()
    wv_d = nc.dram_tensor("wv", [H, DH, D], bf16, kind="ExternalInput").ap()
    y_d = nc.dram_tensor("y", [S, BPC, D], f32, kind="ExternalOutput").ap()
    with tile.TileContext(nc) as tc, ExitStack() as ctx:
        pools = _make_pools(tc, ctx)
        aps = (x_d, kidx_d, kmsk_d, wq_d, wk_d, wv_d, y_d)
        for u in range(unroll):
            _emit(nc, tc, pools, tile, mybir, bass, aps, nkc, u)
    nc.compile()
    return nc


def get_compiled(nkc=DEFAULT_NKC):
    if nkc not in _compiled:
        _compiled[nkc] = _build(nkc=nkc)
    return _compiled[nkc]


def _compute_nkc(mask):
    nb_max = int((np.asarray(mask) != 0).sum(axis=1).max())
    return max(1, -(-nb_max // 128))


def make_in_maps(x, mask, wq, wk, wv, nkc=DEFAULT_NKC):
    import ml_dtypes

    bf = ml_dtypes.bfloat16
    x = np.asarray(x, np.float32)
    mask = np.asarray(mask, np.float32)
    wq_b = np.ascontiguousarray(np.asarray(wq, np.float32).astype(bf))
    wk_b = np.ascontiguousarray(np.asarray(wk, np.float32).astype(bf))
    wv_b = np.ascontiguousarray(np.asarray(wv, np.float32).astype(bf))
    NK = nkc * 128
    maps = []
    for c in range(NCORES):
        mb = mask[c * BPC : (c + 1) * BPC, :]
        kidx = np.zeros((BPC, NK), np.int32)
        kmsk = np.zeros((BPC, NK), np.float32)
        for b in range(BPC):
            valid = np.nonzero(mb[b] != 0)[0][:NK]
            # row index into the per-core x flattened as [(s b), d]
            kidx[b, : len(valid)] = valid.astype(np.int32) * BPC + b
            kmsk[b, : len(valid)] = 1.0
        maps.append(
            {
                "x": np.ascontiguousarray(
                    x[:, c * BPC : (c + 1) * BPC, :].astype(bf)
                ),
                "kidx": kidx,
                "kmsk": kmsk,
                "wq": wq_b,
                "wk": wk_b,
                "wv": wv_b,
            }
        )
    return maps


def kernel(x, mask, wq, wk, wv):
    from concourse.bass_utils import run_bass_kernel_spmd

    nkc = _compute_nkc(mask)
    nc = get_compiled(nkc)
    in_maps = make_in_maps(x, mask, wq, wk, wv, nkc=nkc)
    res = run_bass_kernel_spmd(nc, in_maps, list(range(NCORES))).results
    y = np.concatenate([r["y"] for r in res], axis=1)
    return np.ascontiguousarray(y.astype(np.float32, copy=False))
